# revision 15
# baseline (speedup 1.0000x reference)
"""Trainium2 Bass kernel for nn_MCM_37031208026850 (v17).

Strategy (8 NeuronCores, SPMD):
  - Folded score algebra: score_ij = q_i.k_j = qhat_i.x_j + a_i with
    qhat = (Wq^T Wk)^T x + Wk^T bq. The row-constant a_i and the exact
    mean-over-HW term are host matvecs folded into musum, so the device
    only needs max_j(qhat.x_j) per (row, batch).
  - The k side of every score GEMM is therefore the RAW input x, which the
    host replicates to all 8 cores in bf16 (resident in SBUF) - the two
    1MB-per-rank k AllGathers of v16 are gone. Only two tiny pair-wise
    co AllGathers (2x512 fp32 within each core pair) remain, since the
    softmax gate for batch b couples exactly the rows owned by cores
    2b/2b+1.
  - Scores run bf16 x bf16 (measured 913ns vs 988ns fp32r per 128x512
    tile; fp8 DoubleRow variants were measured and rejected: pure fp8 is
    1.6x faster but rel_err 3e-2 > 2e-2, error-compensated fp8 is slower
    than fp32r). PSUM accumulates fp32; the DVE max-reduces [P,1024]
    groups (one per (mi, combo, batch)) directly into maxd.
  - Shard: core r owns global score rows [512r, 512(r+1)) (batch r//2).
    Projections, values, gating and the three 3x3 convs as in v16, with
    bf16 operands everywhere on the PE; conv PSUM groups cover 16 rows
    (512-wide) to halve instruction-overhead, and the qi=0 gate softmax
    runs inside the score loop so seg3 starts with T1 gating immediately.
  - Timing builds emit each segment body TIME_UNROLL times per For_i
    iteration so the loop-barrier cost is amortized in the measurement.
  - Measured (same-session A/B): v16 271.2us/iter -> v17 160.7-176us/iter
    (regime-dependent); collective round (2 tiny AllGathers) 15.1us.
"""
import sys
sys.path.insert(0, "/opt/trn_rl_repo")

import numpy as np
import ml_dtypes

import concourse.bass as bass
import concourse.mybir as mybir
import concourse.tile as tile
from concourse import bacc
from concourse import bass_utils
from concourse.masks import make_identity

B, C, H, W = 4, 512, 32, 32
HW = H * W
SCALE = 1.0 / C ** 0.5
NCORES = 8
P = 128
KT = C // P          # 4 k-tiles over channels
S = 512              # q-rows per core
BHW = B * HW
TIME_UNROLL = 2      # bodies emitted per For_i iteration in timing builds
F32 = mybir.dt.float32
F32R = mybir.dt.float32r
BF16 = mybir.dt.bfloat16
AX = mybir.AxisListType.X
AF = mybir.ActivationFunctionType
MUL = mybir.AluOpType.mult
ADD = mybir.AluOpType.add

BF = ml_dtypes.bfloat16


def _kmaj(a):
    # (C, n) -> (P, KT, n) k-major layout matching SBUF tiles
    n = a.shape[1]
    return np.ascontiguousarray(
        a.reshape(KT, P, n).transpose(1, 0, 2))


# ----------------------------------------------------------------------------
# host-side preparation
# ----------------------------------------------------------------------------

def host_prep(inputs):
    """Build the 8 per-core input maps from the full problem inputs."""
    xc = np.ascontiguousarray(inputs["xc"], dtype=np.float32)
    xt = np.ascontiguousarray(inputs["xt"], dtype=np.float32)
    f = lambda k: np.ascontiguousarray(inputs[k], dtype=np.float32)
    Wq = [f("Wq_c"), f("Wq_t")]
    bq = [f("bq_c"), f("bq_t")]
    Wk = [f("Wk_c"), f("Wk_t")]
    bk = [f("bk_c"), f("bk_t")]
    Wv_c, bv_c = f("Wv_c"), f("bv_c")
    W64, b64 = f("W512_64"), f("b512_64")
    W1, b1 = f("W1"), f("b1")
    W2, b2 = f("W2"), f("b2")
    W3, b3 = f("W3"), f("b3")

    xcG = np.ascontiguousarray(
        xc.reshape(B, C, HW).transpose(1, 0, 2).reshape(C, BHW))
    xtT = np.ascontiguousarray(
        xt.transpose(2, 0, 1).reshape(C, BHW))
    Xs = [xcG, xtT]

    # exact mean term + row constant a_i, folded into musum:
    #   musum[i, combo] = sum_b mean_j(q_i.k_j) + 4*a_i
    #   with q = Wq x + bq, k = Wk x + bk, a_i = q_i . bk
    kbar = []
    for kk in range(2):
        ks = (Wk[kk] @ Xs[kk].reshape(C, B, HW).sum(-1)
              + HW * bk[kk][:, None]) / HW          # (C, B) per-batch k mean
        kbar.append(ks.sum(1))                      # (C,)
    mus_full = np.zeros((2, 2, BHW), np.float32)
    for qi in range(2):
        for kk in range(2):
            u = Wq[qi].T @ kbar[kk]
            mean_t = Xs[qi].T @ u + float(bq[qi] @ kbar[kk])
            a_i = Xs[qi].T @ (Wq[qi] @ bk[kk]) + float(bq[qi] @ bk[kk])
            mus_full[qi, kk] = mean_t + 4.0 * a_i

    # folded qhat projection weights: qhat = M^T x + c,  M = Wq^T Wk
    mblob = np.concatenate(
        [_kmaj((Wq[qi].T @ Wk[kk]).astype(np.float32))
         for qi in range(2) for kk in range(2)],
        axis=2).astype(BF)                          # (P, KT, 4*512)
    cbias = np.stack(
        [(Wk[kk].T @ bq[qi]).reshape(KT, P).T
         for qi in range(2) for kk in range(2)],
        axis=1)                                     # (P, 4, KT)

    # value weights (folded 64ch c-path), conv weights
    Wcv = W64 @ Wv_c
    bcv = W64 @ bv_c
    wcv64 = _kmaj(np.ascontiguousarray(np.concatenate([Wcv, Wcv], 0).T))
    wtv64 = _kmaj(np.ascontiguousarray(np.concatenate([W64, W64], 0).T))
    w3t = W3.transpose(1, 2, 3, 0).reshape(P, 9 * 64)
    pad = np.zeros((P - 64, 9 * 64), np.float32)
    vblob_shared = np.hstack([
        wcv64.reshape(P, KT * P), wtv64.reshape(P, KT * P),
        W1.transpose(1, 2, 3, 0).reshape(P, 9 * 64),
        W2.transpose(1, 2, 3, 0).reshape(P, 9 * 64),
        np.vstack([w3t[:64], pad]),
        np.vstack([w3t[64:], pad]),
    ]).astype(np.float32)                           # (P, 3328)

    cpad = np.zeros((P - 64, 1), np.float32)
    bblob = np.hstack([
        cbias.reshape(P, 16),
        np.concatenate([bcv, bcv]).reshape(P, 1),
        np.concatenate([b64, b64]).reshape(P, 1),
        np.vstack([b1.reshape(64, 1), cpad]),
        np.vstack([b2.reshape(64, 1), cpad]),
        np.vstack([b3.reshape(64, 1), cpad]),
    ]).astype(np.float32)                           # (P, 21)

    xk_c = _kmaj(xcG).astype(BF).reshape(P, KT * BHW)
    xk_t = _kmaj(xtT).astype(BF).reshape(P, KT * BHW)
    shared = {
        "mblob": np.ascontiguousarray(mblob.reshape(P, KT * 4 * 512)),
        "bblob": np.ascontiguousarray(bblob),
        "xk_c": np.ascontiguousarray(xk_c),
        "xk_t": np.ascontiguousarray(xk_t),
    }

    in_maps = []
    for r in range(NCORES):
        myb = r // 2
        cols = slice(S * r, S * (r + 1))
        bcols = slice(HW * myb, HW * (myb + 1))
        # gate-row one-hots: gates_sb rows are [c2*4 + b]; T1 = [c_co; ct_co]
        # (combos 0,1 of half 0), T2 = [t_co; tc_co] (combos 3,2 of half 1)
        sel1 = np.zeros((8, P), np.float32)
        sel2 = np.zeros((8, P), np.float32)
        for p in range(P):
            sel1[(0 if p < 64 else 1) * 4 + myb, p] = 1.0
            sel2[((3 if p < 64 else 2) - 2) * 4 + myb, p] = 1.0
        selpad = np.zeros((P - 8, P), np.float32)
        vblob = np.hstack([
            vblob_shared,
            np.vstack([sel1, selpad]),
            np.vstack([sel2, selpad]),
        ]).astype(BF)                               # (P, 3584)
        mus = np.zeros((P, KT, 4), np.float32)
        for qi in range(2):
            for kk in range(2):
                mus[:, :, 2 * qi + kk] = (
                    mus_full[qi, kk, cols].reshape(KT, P).T)
        m = dict(shared)
        m["vblob"] = np.ascontiguousarray(vblob)
        m["musum"] = np.ascontiguousarray(mus.reshape(P, KT * 4))
        m["xq_c"] = np.ascontiguousarray(
            _kmaj(xcG[:, cols]).astype(BF).reshape(P, KT * S))
        m["xq_t"] = np.ascontiguousarray(
            _kmaj(xtT[:, cols]).astype(BF).reshape(P, KT * S))
        m["xv_c"] = np.ascontiguousarray(
            _kmaj(xcG[:, bcols]).astype(BF).reshape(P, KT * HW))
        m["xv_t"] = np.ascontiguousarray(
            _kmaj(xtT[:, bcols]).astype(BF).reshape(P, KT * HW))
        in_maps.append(m)
    return in_maps


# ----------------------------------------------------------------------------
# device program
# ----------------------------------------------------------------------------

def build_program(time_reps: int = 1):
    """Build + bacc-compile the SPMD Bass program.

    time_reps > 1 wraps the three compute segments in For_i loops
    (collectives stay outside) so wall-clock deltas between different reps
    counts measure pure per-iteration compute time.
    """
    import contextlib
    nc = bacc.Bacc("TRN2", target_bir_lowering=False, debug=False,
                   num_devices=NCORES)

    def din(name, shape, dtype):
        return nc.dram_tensor(name, list(shape), dtype, kind="ExternalInput")

    xq_d = [din("xq_c", (P, KT * S), BF16), din("xq_t", (P, KT * S), BF16)]
    xk_d = [din("xk_c", (P, KT * BHW), BF16),
            din("xk_t", (P, KT * BHW), BF16)]
    xv_d = [din("xv_c", (P, KT * HW), BF16), din("xv_t", (P, KT * HW), BF16)]
    mblob_d = din("mblob", (P, KT * 4 * 512), BF16)
    VBLOB = 512 + 512 + 576 + 576 + 576 + 576 + 128 + 128
    vblob_d = din("vblob", (P, VBLOB), BF16)
    bblob_d = din("bblob", (P, 21), F32)
    musum_d = din("musum", (P, KT * 4), F32)

    outp_d = nc.dram_tensor("outp", [64, HW], F32, kind="ExternalOutput")


    with tile.TileContext(nc) as tc:
      with tc.tile_pool(name="consts", bufs=1) as cons, \
           tc.tile_pool(name="dram", bufs=1, space="DRAM") as dram:
        # ---------------- static loads (phase-ordered) ----------------
        bblob_sb = cons.tile([P, 21], F32)
        nc.sync.dma_start(bblob_sb, bblob_d.ap())
        xq_sb = [cons.tile([P, KT, S], BF16, name=f"xq{i}") for i in range(2)]
        mblob_sb = cons.tile([P, KT, 4, 512], BF16)
        mb_view = mblob_d.ap().rearrange("p (kt c n) -> p kt c n", kt=KT, c=4)
        for i in range(2):
            nc.sync.dma_start(
                xq_sb[i], xq_d[i].ap().rearrange("p (kt n) -> p kt n", kt=KT))
        for kt in range(KT):
            nc.sync.dma_start(mblob_sb[:, kt], mb_view[:, kt])
        musum_sb = cons.tile([P, KT, 4], F32)
        nc.sync.dma_start(musum_sb, musum_d.ap().rearrange(
            "p (kt c) -> p kt c", kt=KT))
        # k-side raw x, resident bf16, loaded in batch-chunks so the first
        # score groups wait only on their own chunk
        xk_sb = [cons.tile([P, KT, BHW], BF16, name=f"xk{i}")
                 for i in range(2)]
        for i in range(2):
            xk_view = xk_d[i].ap().rearrange("p (kt n) -> p kt n", kt=KT)
            for b in range(B):
                cs = slice(HW * b, HW * (b + 1))
                nc.sync.dma_start(xk_sb[i][:, :, cs], xk_view[:, :, cs])
        vblob_sb = cons.tile([P, VBLOB], BF16)
        nc.sync.dma_start(vblob_sb, vblob_d.ap())
        xv_sb = [cons.tile([P, KT, HW], BF16, name=f"xv{i}") for i in range(2)]
        for i in range(2):
            nc.sync.dma_start(
                xv_sb[i], xv_d[i].ap().rearrange("p (kt n) -> p kt n", kt=KT))

        # blob views
        def vsl(lo, n):
            return vblob_sb[:, lo:lo + n]

        wcv_sb = vsl(0, 512).rearrange("p (kt n) -> p kt n", kt=KT)
        wtv_sb = vsl(512, 512).rearrange("p (kt n) -> p kt n", kt=KT)
        w1t_sb = vsl(1024, 576).rearrange("p (t n) -> p t n", t=9)
        w2t_sb = vsl(1600, 576).rearrange("p (t n) -> p t n", t=9)
        w3a_sb = vblob_sb[0:64, 2176:2752].rearrange("p (t n) -> p t n", t=9)
        w3b_sb = vblob_sb[0:64, 2752:3328].rearrange("p (t n) -> p t n", t=9)
        sel1_sb = vblob_sb[0:8, 3328:3456]
        sel2_sb = vblob_sb[0:8, 3456:3584]
        conv_w = [w1t_sb, w2t_sb]
        cb_sb = bblob_sb[:, 0:16].rearrange("p (c kt) -> p c kt", c=4)
        bcv_sb = bblob_sb[:, 16:17]
        b64_sb = bblob_sb[:, 17:18]
        conv_b = [bblob_sb[0:64, 18:19], bblob_sb[0:64, 19:20]]
        cb3_sb = bblob_sb[0:64, 20:21]

        ident = cons.tile([P, P], F32)
        make_identity(nc, ident)

        # persistent intermediates
        q_sb = [cons.tile([P, KT, S], BF16, name=f"q{i}") for i in range(4)]
        cv_sb = cons.tile([P, HW], F32)
        tv_sb = cons.tile([P, HW], F32)
        maxd = cons.tile([P, KT, 4, 4], F32)       # [i, mi, combo, b]
        sumd = cons.tile([P, KT, 4], F32)
        co_sb = cons.tile([P, KT, 4], F32)         # [i, mi, combo]
        co_row = [cons.tile([2, S], F32, name=f"cor{i}") for i in range(2)]
        gates_sb = [cons.tile([8, HW], F32, name=f"g{i}") for i in range(2)]
        rmax = [cons.tile([8, 1], F32, name=f"rm{i}") for i in range(2)]
        negmax = [cons.tile([8, 1], F32, name=f"nm{i}") for i in range(2)]
        expacc = [cons.tile([8, 1], F32, name=f"ea{i}") for i in range(2)]
        rsum = [cons.tile([8, 1], F32, name=f"rs{i}") for i in range(2)]
        expg = [cons.tile([8, HW], F32, name=f"eg{i}") for i in range(2)]
        gates_n = [cons.tile([8, HW], BF16, name=f"gn{i}") for i in range(2)]
        T1 = cons.tile([P, H + 2, W + 2], BF16)
        T2 = cons.tile([P, H + 2, W + 2], BF16)
        T3a = cons.tile([64, H + 2, W + 2], BF16)
        T3b = cons.tile([64, H + 2, W + 2], BF16)
        out_sb = cons.tile([64, H, W], F32)
        for T in (T1, T2, T3a, T3b):
            nc.vector.memset(T.bitcast(mybir.dt.uint16), 0)

        co_dram = [dram.tile([2, S], F32, name=f"cod{i}") for i in range(2)]
        co_all = [dram.tile([NCORES * 2, S], F32, addr_space="Shared",
                            name=f"coa{i}") for i in range(2)]

        rep = (lambda: tc.For_i(0, time_reps, 1)) if time_reps > 1 else None

        # ------------- segment 1+2: projections, scores + co -------------
        with tc.tile_pool(name="pj", bufs=2, space="PSUM") as pj, \
             tc.tile_pool(name="sc", bufs=2, space="PSUM") as sc, \
             tc.tile_pool(name="vp", bufs=1, space="PSUM") as vp, \
             tc.tile_pool(name="fin", bufs=1, space="PSUM") as fin:
          with rep() if rep else contextlib.nullcontext():
           for _u in range(TIME_UNROLL if time_reps > 1 else 1):
            evac_i = 0
            for combo in range(4):
                qi = combo // 2
                for m in range(KT):
                    pq = pj.tile([P, S], F32, tag="pq", name="pq")
                    for kt in range(KT):
                        nc.tensor.matmul(
                            pq,
                            mblob_sb[:, kt, combo, P * m:P * (m + 1)],
                            xq_sb[qi][:, kt], start=(kt == 0),
                            stop=(kt == KT - 1))
                    if evac_i % 2 == 0:
                        nc.vector.tensor_scalar_add(
                            q_sb[combo][:, m, :], pq, cb_sb[:, combo, m:m + 1])
                    else:
                        nc.scalar.activation(
                            q_sb[combo][:, m, :], pq, AF.Identity,
                            bias=cb_sb[:, combo, m:m + 1])
                    evac_i += 1

            def score_group(combo, b, mi):
                kk = combo % 2
                ps = sc.tile([P, 1024], F32, tag="ps", name="ps")
                for h_ in range(2):
                    for kt in range(KT):
                        nc.tensor.matmul(
                            ps[:, 512 * h_:512 * (h_ + 1)],
                            q_sb[combo][:, kt, P * mi:P * (mi + 1)],
                            xk_sb[kk][:, kt,
                                      HW * b + 512 * h_:HW * b + 512 * (h_ + 1)],
                            start=(kt == 0), stop=(kt == KT - 1))
                nc.vector.reduce_max(
                    maxd[:, mi, combo, b:b + 1], ps, axis=AX)

            def co_half(qi):
                sl = slice(2 * qi, 2 * qi + 2)
                nc.vector.reduce_sum(sumd[:, :, sl], maxd[:, :, sl], axis=AX)
                nc.vector.tensor_tensor(co_sb[:, :, sl], sumd[:, :, sl],
                                        musum_sb[:, :, sl], ADD)
                for mi in range(KT):
                    ptr = fin.tile([P, P], F32, tag="ptr", name="ptr")
                    nc.tensor.transpose(ptr[:2, :], co_sb[:, mi, sl], ident)
                    nc.vector.tensor_copy(
                        co_row[qi][:, P * mi:P * (mi + 1)], ptr[:2, :])
                nc.sync.dma_start(co_dram[qi].opt(), co_row[qi])
                if time_reps <= 1:
                    nc.gpsimd.collective_compute(
                        "AllGather", mybir.AluOpType.bypass,
                        replica_groups=[list(range(NCORES))],
                        ins=[co_dram[qi].opt()], outs=[co_all[qi].opt()])

            def gates_half(qi):
                # gates rows (cmb in half qi) x batch, softmaxed
                co_view = co_all[qi].opt().rearrange(
                    "(b h c) i -> c b h i", b=4, h=2, c=2)
                for c2 in range(2):
                    nc.sync.dma_start(
                        gates_sb[qi][4 * c2:4 * (c2 + 1), :].rearrange(
                            "p (h i) -> p h i", h=2),
                        co_view[c2])
                nc.vector.reduce_max(rmax[qi], gates_sb[qi], axis=AX)
                nc.vector.tensor_scalar_mul(negmax[qi], rmax[qi], -SCALE)
                nc.scalar.activation(expg[qi], gates_sb[qi], AF.Exp,
                                     bias=negmax[qi], scale=SCALE,
                                     accum_out=expacc[qi])
                nc.vector.reciprocal(rsum[qi], expacc[qi])
                nc.vector.tensor_scalar_mul(gates_n[qi], expg[qi], rsum[qi])

            for qi in range(2):
                for combo in (2 * qi, 2 * qi + 1):
                    for b in range(B):
                        for mi in range(KT):
                            score_group(combo, b, mi)
                if qi == 1:
                    # folded 64-ch value projections fill the PE drain gap
                    for vi, (wv, vt) in enumerate(((wcv_sb, cv_sb),
                                                   (wtv_sb, tv_sb))):
                        for nh in range(2):
                            pv = vp.tile([P, 512], F32, tag="pv", name="pv")
                            for kt in range(KT):
                                nc.tensor.matmul(
                                    pv, wv[:, kt],
                                    xv_sb[vi][:, kt,
                                              512 * nh:512 * (nh + 1)],
                                    start=(kt == 0), stop=(kt == KT - 1))
                            if vi == 0:
                                nc.scalar.activation(
                                    vt[:, 512 * nh:512 * (nh + 1)], pv,
                                    AF.Identity, bias=bcv_sb)
                            else:
                                nc.scalar.copy(
                                    vt[:, 512 * nh:512 * (nh + 1)], pv)
                co_half(qi)
                if qi == 0:
                    gates_half(0)

        if time_reps > 1:
            for qi in range(2):
                nc.gpsimd.collective_compute(
                    "AllGather", mybir.AluOpType.bypass,
                    replica_groups=[list(range(NCORES))],
                    ins=[co_dram[qi].opt()], outs=[co_all[qi].opt()])

        # ---------------- segment 3: gates + fusion convs ----------------
        with tc.tile_pool(name="g", bufs=2, space="PSUM") as g:
          with rep() if rep else contextlib.nullcontext():
           for _u in range(TIME_UNROLL if time_reps > 1 else 1):
            def gate_one(sel, gn, val, T):
                for nh in range(2):
                    pbg = g.tile([P, 512], F32, tag="pbg", name="pbg")
                    nc.tensor.matmul(pbg, sel,
                                     gn[:, 512 * nh:512 * (nh + 1)],
                                     start=True, stop=True)
                    reg = T[:, 1 + 16 * nh:17 + 16 * nh, 1:33]
                    nc.vector.tensor_tensor(
                        reg, pbg.rearrange("p (y x) -> p y x", y=16),
                        val[:, 512 * nh:512 * (nh + 1)].rearrange(
                            "p (y x) -> p y x", y=16), MUL)
                    nc.vector.tensor_scalar_add(reg, reg, b64_sb)

            def conv12(srcT, wi, dstT):
                for cy in range(2):
                    pc = g.tile([64, 16, 32], F32, tag="pc", name="pc")
                    for tap in range(9):
                        dy, dx = tap // 3, tap % 3
                        nc.tensor.matmul(
                            pc, conv_w[wi][:, tap, :],
                            srcT[:, 16 * cy + dy:16 * cy + dy + 16,
                                 dx:dx + 32],
                            start=(tap == 0), stop=(tap == 8))
                    nc.scalar.activation(
                        dstT[:, 1 + 16 * cy:17 + 16 * cy, 1:33], pc, AF.Relu,
                        bias=conv_b[wi], scale=1.0)

            gate_one(sel1_sb, gates_n[0], cv_sb, T1)
            gates_half(1)
            conv12(T1, 0, T3a)
            gate_one(sel2_sb, gates_n[1], tv_sb, T2)
            conv12(T2, 1, T3b)
            for cy in range(2):
                pc = g.tile([64, 16, 32], F32, tag="pc", name="pc")
                for hi, (wh, Th) in enumerate(((w3a_sb, T3a), (w3b_sb, T3b))):
                    for tap in range(9):
                        dy, dx = tap // 3, tap % 3
                        nc.tensor.matmul(
                            pc, wh[:, tap, :],
                            Th[:, 16 * cy + dy:16 * cy + dy + 16,
                               dx:dx + 32],
                            start=(hi == 0 and tap == 0),
                            stop=(hi == 1 and tap == 8))
                nc.scalar.activation(out_sb[:, 16 * cy:16 * (cy + 1), :], pc,
                                     AF.Relu, bias=cb3_sb, scale=1.0)
                nc.sync.dma_start(
                    outp_d.ap().rearrange("o (y x) -> o y x",
                                          y=H)[:, 16 * cy:16 * (cy + 1), :],
                    out_sb[:, 16 * cy:16 * (cy + 1), :])

    nc.compile()
    return nc


# ----------------------------------------------------------------------------
# entry point
# ----------------------------------------------------------------------------

_CACHE = {}


def _get_nc():
    if "nc" not in _CACHE:
        _CACHE["nc"] = build_program()
    return _CACHE["nc"]


def kernel(**inputs) -> np.ndarray:
    nc = _get_nc()
    in_maps = host_prep(inputs)
    res = bass_utils.run_bass_kernel_spmd(nc, in_maps,
                                          core_ids=list(range(NCORES)))
    out = np.empty((B, 64, H, W), np.float32)
    for b in range(B):
        out[b] = res.results[2 * b]["outp"].reshape(64, H, W)
    return out


if __name__ == "__main__":
    rng = np.random.default_rng(0)
    d = {
        "xc": rng.standard_normal((B, C, H, W), np.float32),
        "xt": rng.standard_normal((B, HW, C), np.float32),
    }
    for nm, o in (("q_c", C), ("k_c", C), ("v_c", C), ("q_t", C), ("k_t", C)):
        d[f"W{nm}"] = rng.standard_normal((o, C), np.float32) * 0.02
        d[f"b{nm}"] = np.zeros(o, np.float32)
    d["W512_64"] = rng.standard_normal((64, C), np.float32) * 0.02
    d["b512_64"] = np.zeros(64, np.float32)
    for i in (1, 2, 3):
        d[f"W{i}"] = rng.standard_normal((64, 128, 3, 3), np.float32) * 0.02
        d[f"b{i}"] = np.zeros(64, np.float32)
    out = kernel(**d)
    print("out", out.shape, out.dtype, np.abs(out).max())


# revision 16
# speedup vs baseline: 1.1173x; 1.1173x over previous
"""Trainium2 Bass kernel for nn_MCM_37031208026850 (v17).

Strategy (8 NeuronCores, SPMD):
  - Folded score algebra: score_ij = q_i.k_j = qhat_i.x_j + a_i with
    qhat = (Wq^T Wk)^T x + Wk^T bq. The row-constant a_i and the exact
    mean-over-HW term are host matvecs folded into musum, so the device
    only needs max_j(qhat.x_j) per (row, batch).
  - The k side of every score GEMM is therefore the RAW input x, which the
    host replicates to all 8 cores in bf16 (resident in SBUF) - the two
    1MB-per-rank k AllGathers of v16 are gone. Only two tiny pair-wise
    co AllGathers (2x512 fp32 within each core pair) remain, since the
    softmax gate for batch b couples exactly the rows owned by cores
    2b/2b+1.
  - Scores run bf16 x bf16 (measured 913ns vs 988ns fp32r per 128x512
    tile; fp8 DoubleRow variants were measured and rejected: pure fp8 is
    1.6x faster but rel_err 3e-2 > 2e-2, error-compensated fp8 is slower
    than fp32r). PSUM accumulates fp32; the DVE max-reduces [P,1024]
    groups (one per (mi, combo, batch)) directly into maxd.
  - Shard: core r owns global score rows [512r, 512(r+1)) (batch r//2).
    Projections, values, gating and the three 3x3 convs as in v16, with
    bf16 operands everywhere on the PE; conv PSUM groups cover 16 rows
    (512-wide) to halve instruction-overhead, and the qi=0 gate softmax
    runs inside the score loop so seg3 starts with T1 gating immediately.
  - Timing builds emit each segment body TIME_UNROLL times per For_i
    iteration so the loop-barrier cost is amortized in the measurement.
  - Measured (same-session A/B): v16 271.2us/iter -> v17 160.7-176us/iter
    (regime-dependent); collective round (2 tiny AllGathers) 15.1us.
"""
import sys
sys.path.insert(0, "/opt/trn_rl_repo")

import numpy as np
import ml_dtypes

import concourse.bass as bass
import concourse.mybir as mybir
import concourse.tile as tile
from concourse import bacc
from concourse import bass_utils
from concourse.masks import make_identity

B, C, H, W = 4, 512, 32, 32
HW = H * W
SCALE = 1.0 / C ** 0.5
NCORES = 8
P = 128
KT = C // P          # 4 k-tiles over channels
S = 512              # q-rows per core
BHW = B * HW
TIME_UNROLL = 2      # bodies emitted per For_i iteration in timing builds
F32 = mybir.dt.float32
F32R = mybir.dt.float32r
BF16 = mybir.dt.bfloat16
AX = mybir.AxisListType.X
AF = mybir.ActivationFunctionType
MUL = mybir.AluOpType.mult
ADD = mybir.AluOpType.add

BF = ml_dtypes.bfloat16


def _kmaj(a):
    # (C, n) -> (P, KT, n) k-major layout matching SBUF tiles
    n = a.shape[1]
    return np.ascontiguousarray(
        a.reshape(KT, P, n).transpose(1, 0, 2))


# ----------------------------------------------------------------------------
# host-side preparation
# ----------------------------------------------------------------------------

def host_prep(inputs):
    """Build the 8 per-core input maps from the full problem inputs."""
    xc = np.ascontiguousarray(inputs["xc"], dtype=np.float32)
    xt = np.ascontiguousarray(inputs["xt"], dtype=np.float32)
    f = lambda k: np.ascontiguousarray(inputs[k], dtype=np.float32)
    Wq = [f("Wq_c"), f("Wq_t")]
    bq = [f("bq_c"), f("bq_t")]
    Wk = [f("Wk_c"), f("Wk_t")]
    bk = [f("bk_c"), f("bk_t")]
    Wv_c, bv_c = f("Wv_c"), f("bv_c")
    W64, b64 = f("W512_64"), f("b512_64")
    W1, b1 = f("W1"), f("b1")
    W2, b2 = f("W2"), f("b2")
    W3, b3 = f("W3"), f("b3")

    xcG = np.ascontiguousarray(
        xc.reshape(B, C, HW).transpose(1, 0, 2).reshape(C, BHW))
    xtT = np.ascontiguousarray(
        xt.transpose(2, 0, 1).reshape(C, BHW))
    Xs = [xcG, xtT]

    # exact mean term + row constant a_i, folded into musum:
    #   musum[i, combo] = sum_b mean_j(q_i.k_j) + 4*a_i
    #   with q = Wq x + bq, k = Wk x + bk, a_i = q_i . bk
    kbar = []
    for kk in range(2):
        ks = (Wk[kk] @ Xs[kk].reshape(C, B, HW).sum(-1)
              + HW * bk[kk][:, None]) / HW          # (C, B) per-batch k mean
        kbar.append(ks.sum(1))                      # (C,)
    mus_full = np.zeros((2, 2, BHW), np.float32)
    for qi in range(2):
        for kk in range(2):
            u = Wq[qi].T @ kbar[kk]
            mean_t = Xs[qi].T @ u + float(bq[qi] @ kbar[kk])
            a_i = Xs[qi].T @ (Wq[qi] @ bk[kk]) + float(bq[qi] @ bk[kk])
            mus_full[qi, kk] = mean_t + 4.0 * a_i

    # folded qhat projection weights: qhat = M^T x + c,  M = Wq^T Wk
    mblob = np.concatenate(
        [_kmaj((Wq[qi].T @ Wk[kk]).astype(np.float32))
         for qi in range(2) for kk in range(2)],
        axis=2).astype(BF)                          # (P, KT, 4*512)
    cbias = np.stack(
        [(Wk[kk].T @ bq[qi]).reshape(KT, P).T
         for qi in range(2) for kk in range(2)],
        axis=1)                                     # (P, 4, KT)

    # value weights (folded 64ch c-path), conv weights
    Wcv = W64 @ Wv_c
    bcv = W64 @ bv_c
    wcv64 = _kmaj(np.ascontiguousarray(np.concatenate([Wcv, Wcv], 0).T))
    wtv64 = _kmaj(np.ascontiguousarray(np.concatenate([W64, W64], 0).T))
    w3t = W3.transpose(1, 2, 3, 0).reshape(P, 9 * 64)
    pad = np.zeros((P - 64, 9 * 64), np.float32)
    vblob_shared = np.hstack([
        wcv64.reshape(P, KT * P), wtv64.reshape(P, KT * P),
        W1.transpose(1, 2, 3, 0).reshape(P, 9 * 64),
        W2.transpose(1, 2, 3, 0).reshape(P, 9 * 64),
        np.vstack([w3t[:64], pad]),
        np.vstack([w3t[64:], pad]),
    ]).astype(np.float32)                           # (P, 3328)

    cpad = np.zeros((P - 64, 1), np.float32)
    bblob = np.hstack([
        cbias.reshape(P, 16),
        np.concatenate([bcv, bcv]).reshape(P, 1),
        np.concatenate([b64, b64]).reshape(P, 1),
        np.vstack([b1.reshape(64, 1), cpad]),
        np.vstack([b2.reshape(64, 1), cpad]),
        np.vstack([b3.reshape(64, 1), cpad]),
    ]).astype(np.float32)                           # (P, 21)

    xk_c = _kmaj(xcG).astype(BF).reshape(P, KT * BHW)
    xk_t = _kmaj(xtT).astype(BF).reshape(P, KT * BHW)
    F8 = ml_dtypes.float8_e4m3
    xk8_c = np.concatenate(
        [_kmaj(xcG[:, HW * b:HW * (b + 1)]).astype(F8) for b in (0, 2)],
        axis=2).reshape(P, KT * 2 * HW)
    xk8_t = np.concatenate(
        [_kmaj(xtT[:, HW * b:HW * (b + 1)]).astype(F8) for b in (1, 3)],
        axis=2).reshape(P, KT * 2 * HW)
    shared = {
        "mblob": np.ascontiguousarray(mblob.reshape(P, KT * 4 * 512)),
        "bblob": np.ascontiguousarray(bblob),
        "xk_c": np.ascontiguousarray(xk_c),
        "xk_t": np.ascontiguousarray(xk_t),
        "xk8_c": np.ascontiguousarray(xk8_c),
        "xk8_t": np.ascontiguousarray(xk8_t),
    }

    in_maps = []
    for r in range(NCORES):
        myb = r // 2
        cols = slice(S * r, S * (r + 1))
        bcols = slice(HW * myb, HW * (myb + 1))
        # gate-row one-hots: gates_sb rows are [c2*4 + b]; T1 = [c_co; ct_co]
        # (combos 0,1 of half 0), T2 = [t_co; tc_co] (combos 3,2 of half 1)
        sel1 = np.zeros((8, P), np.float32)
        sel2 = np.zeros((8, P), np.float32)
        for p in range(P):
            sel1[(0 if p < 64 else 1) * 4 + myb, p] = 1.0
            sel2[((3 if p < 64 else 2) - 2) * 4 + myb, p] = 1.0
        selpad = np.zeros((P - 8, P), np.float32)
        vblob = np.hstack([
            vblob_shared,
            np.vstack([sel1, selpad]),
            np.vstack([sel2, selpad]),
        ]).astype(BF)                               # (P, 3584)
        mus = np.zeros((P, KT, 4), np.float32)
        for qi in range(2):
            for kk in range(2):
                mus[:, :, 2 * qi + kk] = (
                    mus_full[qi, kk, cols].reshape(KT, P).T)
        m = dict(shared)
        m["vblob"] = np.ascontiguousarray(vblob)
        m["musum"] = np.ascontiguousarray(mus.reshape(P, KT * 4))
        m["xq_c"] = np.ascontiguousarray(
            _kmaj(xcG[:, cols]).astype(BF).reshape(P, KT * S))
        m["xq_t"] = np.ascontiguousarray(
            _kmaj(xtT[:, cols]).astype(BF).reshape(P, KT * S))
        m["xv_c"] = np.ascontiguousarray(
            _kmaj(xcG[:, bcols]).astype(BF).reshape(P, KT * HW))
        m["xv_t"] = np.ascontiguousarray(
            _kmaj(xtT[:, bcols]).astype(BF).reshape(P, KT * HW))
        in_maps.append(m)
    return in_maps


# ----------------------------------------------------------------------------
# device program
# ----------------------------------------------------------------------------

def build_program(time_reps: int = 1):
    """Build + bacc-compile the SPMD Bass program.

    time_reps > 1 wraps the three compute segments in For_i loops
    (collectives stay outside) so wall-clock deltas between different reps
    counts measure pure per-iteration compute time.
    """
    import contextlib
    nc = bacc.Bacc("TRN2", target_bir_lowering=False, debug=False,
                   num_devices=NCORES)

    def din(name, shape, dtype):
        return nc.dram_tensor(name, list(shape), dtype, kind="ExternalInput")

    xq_d = [din("xq_c", (P, KT * S), BF16), din("xq_t", (P, KT * S), BF16)]
    xk_d = [din("xk_c", (P, KT * BHW), BF16),
            din("xk_t", (P, KT * BHW), BF16)]
    FP8 = mybir.dt.float8e4
    xk8_d = [din("xk8_c", (P, KT * 2 * HW), FP8),
             din("xk8_t", (P, KT * 2 * HW), FP8)]
    xv_d = [din("xv_c", (P, KT * HW), BF16), din("xv_t", (P, KT * HW), BF16)]
    mblob_d = din("mblob", (P, KT * 4 * 512), BF16)
    VBLOB = 512 + 512 + 576 + 576 + 576 + 576 + 128 + 128
    vblob_d = din("vblob", (P, VBLOB), BF16)
    bblob_d = din("bblob", (P, 21), F32)
    musum_d = din("musum", (P, KT * 4), F32)

    outp_d = nc.dram_tensor("outp", [64, HW], F32, kind="ExternalOutput")


    with tile.TileContext(nc) as tc:
      with tc.tile_pool(name="consts", bufs=1) as cons, \
           tc.tile_pool(name="dram", bufs=1, space="DRAM") as dram:
        # ---------------- static loads (phase-ordered) ----------------
        bblob_sb = cons.tile([P, 21], F32)
        nc.sync.dma_start(bblob_sb, bblob_d.ap())
        xq_sb = [cons.tile([P, KT, S], BF16, name=f"xq{i}") for i in range(2)]
        mblob_sb = cons.tile([P, KT, 4, 512], BF16)
        mb_view = mblob_d.ap().rearrange("p (kt c n) -> p kt c n", kt=KT, c=4)
        for i in range(2):
            nc.sync.dma_start(
                xq_sb[i], xq_d[i].ap().rearrange("p (kt n) -> p kt n", kt=KT))
        for kt in range(KT):
            nc.sync.dma_start(mblob_sb[:, kt], mb_view[:, kt])
        musum_sb = cons.tile([P, KT, 4], F32)
        nc.sync.dma_start(musum_sb, musum_d.ap().rearrange(
            "p (kt c) -> p kt c", kt=KT))
        # k-side raw x, resident bf16, loaded in batch-chunks so the first
        # score groups wait only on their own chunk
        xk_sb = [cons.tile([P, KT, BHW], BF16, name=f"xk{i}")
                 for i in range(2)]
        for i in range(2):
            xk_view = xk_d[i].ap().rearrange("p (kt n) -> p kt n", kt=KT)
            for b in range(B):
                cs = slice(HW * b, HW * (b + 1))
                nc.sync.dma_start(xk_sb[i][:, :, cs], xk_view[:, :, cs])
        xk8_sb = [cons.tile([P, KT, 2 * HW], FP8, name=f"xk8{i}")
                  for i in range(2)]
        for i in range(2):
            nc.sync.dma_start(
                xk8_sb[i],
                xk8_d[i].ap().rearrange("p (kt n) -> p kt n", kt=KT))
        vblob_sb = cons.tile([P, VBLOB], BF16)
        nc.sync.dma_start(vblob_sb, vblob_d.ap())
        xv_sb = [cons.tile([P, KT, HW], BF16, name=f"xv{i}") for i in range(2)]
        for i in range(2):
            nc.sync.dma_start(
                xv_sb[i], xv_d[i].ap().rearrange("p (kt n) -> p kt n", kt=KT))

        # blob views
        def vsl(lo, n):
            return vblob_sb[:, lo:lo + n]

        wcv_sb = vsl(0, 512).rearrange("p (kt n) -> p kt n", kt=KT)
        wtv_sb = vsl(512, 512).rearrange("p (kt n) -> p kt n", kt=KT)
        w1t_sb = vsl(1024, 576).rearrange("p (t n) -> p t n", t=9)
        w2t_sb = vsl(1600, 576).rearrange("p (t n) -> p t n", t=9)
        w3a_sb = vblob_sb[0:64, 2176:2752].rearrange("p (t n) -> p t n", t=9)
        w3b_sb = vblob_sb[0:64, 2752:3328].rearrange("p (t n) -> p t n", t=9)
        sel1_sb = vblob_sb[0:8, 3328:3456]
        sel2_sb = vblob_sb[0:8, 3456:3584]
        conv_w = [w1t_sb, w2t_sb]
        cb_sb = bblob_sb[:, 0:16].rearrange("p (c kt) -> p c kt", c=4)
        bcv_sb = bblob_sb[:, 16:17]
        b64_sb = bblob_sb[:, 17:18]
        conv_b = [bblob_sb[0:64, 18:19], bblob_sb[0:64, 19:20]]
        cb3_sb = bblob_sb[0:64, 20:21]

        ident = cons.tile([P, P], F32)
        make_identity(nc, ident)

        # persistent intermediates
        q_sb = [cons.tile([P, KT, S], BF16, name=f"q{i}") for i in range(4)]
        q8_sb = [cons.tile([P, KT, S], FP8, name=f"q8{i}") for i in range(4)]
        cv_sb = cons.tile([P, HW], F32)
        tv_sb = cons.tile([P, HW], F32)
        maxd = cons.tile([P, KT, 4, 4], F32)       # [i, mi, combo, b]
        sumd = cons.tile([P, KT, 4], F32)
        co_sb = cons.tile([P, KT, 4], F32)         # [i, mi, combo]
        co_row = [cons.tile([2, S], F32, name=f"cor{i}") for i in range(2)]
        gates_sb = [cons.tile([8, HW], F32, name=f"g{i}") for i in range(2)]
        rmax = [cons.tile([8, 1], F32, name=f"rm{i}") for i in range(2)]
        negmax = [cons.tile([8, 1], F32, name=f"nm{i}") for i in range(2)]
        expacc = [cons.tile([8, 1], F32, name=f"ea{i}") for i in range(2)]
        rsum = [cons.tile([8, 1], F32, name=f"rs{i}") for i in range(2)]
        expg = [cons.tile([8, HW], F32, name=f"eg{i}") for i in range(2)]
        gates_n = [cons.tile([8, HW], BF16, name=f"gn{i}") for i in range(2)]
        T1 = cons.tile([P, H + 2, W + 2], BF16)
        T2 = cons.tile([P, H + 2, W + 2], BF16)
        T3a = cons.tile([64, H + 2, W + 2], BF16)
        T3b = cons.tile([64, H + 2, W + 2], BF16)
        out_sb = cons.tile([64, H, W], F32)
        for T in (T1, T2, T3a, T3b):
            nc.vector.memset(T.bitcast(mybir.dt.uint16), 0)

        co_dram = [dram.tile([2, S], F32, name=f"cod{i}") for i in range(2)]
        co_all = [dram.tile([NCORES * 2, S], F32, addr_space="Shared",
                            name=f"coa{i}") for i in range(2)]

        rep = (lambda: tc.For_i(0, time_reps, 1)) if time_reps > 1 else None

        # ------------- segment 1+2: projections, scores + co -------------
        with tc.tile_pool(name="pj", bufs=2, space="PSUM") as pj, \
             tc.tile_pool(name="sc", bufs=2, space="PSUM") as sc, \
             tc.tile_pool(name="vp", bufs=1, space="PSUM") as vp, \
             tc.tile_pool(name="fin", bufs=1, space="PSUM") as fin:
          with rep() if rep else contextlib.nullcontext():
           for _u in range(TIME_UNROLL if time_reps > 1 else 1):
            evac_i = 0
            for combo in range(4):
                qi = combo // 2
                for m in range(KT):
                    pq = pj.tile([P, S], F32, tag="pq", name="pq")
                    for kt in range(KT):
                        nc.tensor.matmul(
                            pq,
                            mblob_sb[:, kt, combo, P * m:P * (m + 1)],
                            xq_sb[qi][:, kt], start=(kt == 0),
                            stop=(kt == KT - 1))
                    if evac_i % 2 == 0:
                        nc.vector.tensor_scalar_add(
                            q_sb[combo][:, m, :], pq, cb_sb[:, combo, m:m + 1])
                        nc.scalar.activation(
                            q8_sb[combo][:, m, :], pq, AF.Identity,
                            bias=cb_sb[:, combo, m:m + 1])
                    else:
                        nc.scalar.activation(
                            q_sb[combo][:, m, :], pq, AF.Identity,
                            bias=cb_sb[:, combo, m:m + 1])
                        nc.vector.tensor_scalar_add(
                            q8_sb[combo][:, m, :], pq, cb_sb[:, combo, m:m + 1])
                    evac_i += 1

            DRM = mybir.MatmulPerfMode.DoubleRow

            def score_group(combo, b, mi):
                kk = combo % 2
                ps = sc.tile([P, 1024], F32, tag="ps", name="ps")
                if b == combo:
                    # this batch's max runs in pure-fp8 DoubleRow (1 of 4
                    # max terms per combo; rel-err budget measured on
                    # device). xk8 column base: batch slot b//2.
                    c0 = HW * (b // 2)
                    for cb4 in range(4):
                        for slab in range(2):
                            nc.tensor.matmul(
                                ps[:, 256 * cb4:256 * (cb4 + 1)],
                                q8_sb[combo][:, 2 * slab:2 * slab + 2,
                                             P * mi:P * (mi + 1)],
                                xk8_sb[kk][:, 2 * slab:2 * slab + 2,
                                           c0 + 256 * cb4:
                                           c0 + 256 * (cb4 + 1)],
                                start=(slab == 0), stop=(slab == 1),
                                perf_mode=DRM)
                else:
                    for h_ in range(2):
                        for kt in range(KT):
                            nc.tensor.matmul(
                                ps[:, 512 * h_:512 * (h_ + 1)],
                                q_sb[combo][:, kt, P * mi:P * (mi + 1)],
                                xk_sb[kk][:, kt,
                                          HW * b + 512 * h_:
                                          HW * b + 512 * (h_ + 1)],
                                start=(kt == 0), stop=(kt == KT - 1))
                nc.vector.reduce_max(
                    maxd[:, mi, combo, b:b + 1], ps, axis=AX)

            def co_half(qi):
                sl = slice(2 * qi, 2 * qi + 2)
                nc.vector.reduce_sum(sumd[:, :, sl], maxd[:, :, sl], axis=AX)
                nc.vector.tensor_tensor(co_sb[:, :, sl], sumd[:, :, sl],
                                        musum_sb[:, :, sl], ADD)
                for mi in range(KT):
                    ptr = fin.tile([P, P], F32, tag="ptr", name="ptr")
                    nc.tensor.transpose(ptr[:2, :], co_sb[:, mi, sl], ident)
                    nc.vector.tensor_copy(
                        co_row[qi][:, P * mi:P * (mi + 1)], ptr[:2, :])
                nc.sync.dma_start(co_dram[qi].opt(), co_row[qi])
                if time_reps <= 1:
                    nc.gpsimd.collective_compute(
                        "AllGather", mybir.AluOpType.bypass,
                        replica_groups=[list(range(NCORES))],
                        ins=[co_dram[qi].opt()], outs=[co_all[qi].opt()])

            def gates_half(qi):
                # gates rows (cmb in half qi) x batch, softmaxed
                co_view = co_all[qi].opt().rearrange(
                    "(b h c) i -> c b h i", b=4, h=2, c=2)
                for c2 in range(2):
                    nc.sync.dma_start(
                        gates_sb[qi][4 * c2:4 * (c2 + 1), :].rearrange(
                            "p (h i) -> p h i", h=2),
                        co_view[c2])
                nc.vector.reduce_max(rmax[qi], gates_sb[qi], axis=AX)
                nc.vector.tensor_scalar_mul(negmax[qi], rmax[qi], -SCALE)
                nc.scalar.activation(expg[qi], gates_sb[qi], AF.Exp,
                                     bias=negmax[qi], scale=SCALE,
                                     accum_out=expacc[qi])
                nc.vector.reciprocal(rsum[qi], expacc[qi])
                nc.vector.tensor_scalar_mul(gates_n[qi], expg[qi], rsum[qi])

            for qi in range(2):
                for combo in (2 * qi, 2 * qi + 1):
                    for b in range(B):
                        for mi in range(KT):
                            score_group(combo, b, mi)
                if qi == 1:
                    # folded 64-ch value projections fill the PE drain gap
                    for vi, (wv, vt) in enumerate(((wcv_sb, cv_sb),
                                                   (wtv_sb, tv_sb))):
                        for nh in range(2):
                            pv = vp.tile([P, 512], F32, tag="pv", name="pv")
                            for kt in range(KT):
                                nc.tensor.matmul(
                                    pv, wv[:, kt],
                                    xv_sb[vi][:, kt,
                                              512 * nh:512 * (nh + 1)],
                                    start=(kt == 0), stop=(kt == KT - 1))
                            if vi == 0:
                                nc.scalar.activation(
                                    vt[:, 512 * nh:512 * (nh + 1)], pv,
                                    AF.Identity, bias=bcv_sb)
                            else:
                                nc.scalar.copy(
                                    vt[:, 512 * nh:512 * (nh + 1)], pv)
                co_half(qi)
                if qi == 0:
                    gates_half(0)

        if time_reps > 1:
            for qi in range(2):
                nc.gpsimd.collective_compute(
                    "AllGather", mybir.AluOpType.bypass,
                    replica_groups=[list(range(NCORES))],
                    ins=[co_dram[qi].opt()], outs=[co_all[qi].opt()])

        # ---------------- segment 3: gates + fusion convs ----------------
        with tc.tile_pool(name="g", bufs=2, space="PSUM") as g:
          with rep() if rep else contextlib.nullcontext():
           for _u in range(TIME_UNROLL if time_reps > 1 else 1):
            def gate_one(sel, gn, val, T):
                for nh in range(2):
                    pbg = g.tile([P, 512], F32, tag="pbg", name="pbg")
                    nc.tensor.matmul(pbg, sel,
                                     gn[:, 512 * nh:512 * (nh + 1)],
                                     start=True, stop=True)
                    reg = T[:, 1 + 16 * nh:17 + 16 * nh, 1:33]
                    nc.vector.tensor_tensor(
                        reg, pbg.rearrange("p (y x) -> p y x", y=16),
                        val[:, 512 * nh:512 * (nh + 1)].rearrange(
                            "p (y x) -> p y x", y=16), MUL)
                    nc.vector.tensor_scalar_add(reg, reg, b64_sb)

            def conv12(srcT, wi, dstT):
                for cy in range(2):
                    pc = g.tile([64, 16, 32], F32, tag="pc", name="pc")
                    for tap in range(9):
                        dy, dx = tap // 3, tap % 3
                        nc.tensor.matmul(
                            pc, conv_w[wi][:, tap, :],
                            srcT[:, 16 * cy + dy:16 * cy + dy + 16,
                                 dx:dx + 32],
                            start=(tap == 0), stop=(tap == 8))
                    nc.scalar.activation(
                        dstT[:, 1 + 16 * cy:17 + 16 * cy, 1:33], pc, AF.Relu,
                        bias=conv_b[wi], scale=1.0)

            gate_one(sel1_sb, gates_n[0], cv_sb, T1)
            gates_half(1)
            conv12(T1, 0, T3a)
            gate_one(sel2_sb, gates_n[1], tv_sb, T2)
            conv12(T2, 1, T3b)
            for cy in range(2):
                pc = g.tile([64, 16, 32], F32, tag="pc", name="pc")
                for hi, (wh, Th) in enumerate(((w3a_sb, T3a), (w3b_sb, T3b))):
                    for tap in range(9):
                        dy, dx = tap // 3, tap % 3
                        nc.tensor.matmul(
                            pc, wh[:, tap, :],
                            Th[:, 16 * cy + dy:16 * cy + dy + 16,
                               dx:dx + 32],
                            start=(hi == 0 and tap == 0),
                            stop=(hi == 1 and tap == 8))
                nc.scalar.activation(out_sb[:, 16 * cy:16 * (cy + 1), :], pc,
                                     AF.Relu, bias=cb3_sb, scale=1.0)
                nc.sync.dma_start(
                    outp_d.ap().rearrange("o (y x) -> o y x",
                                          y=H)[:, 16 * cy:16 * (cy + 1), :],
                    out_sb[:, 16 * cy:16 * (cy + 1), :])

    nc.compile()
    return nc


# ----------------------------------------------------------------------------
# entry point
# ----------------------------------------------------------------------------

_CACHE = {}


def _get_nc():
    if "nc" not in _CACHE:
        _CACHE["nc"] = build_program()
    return _CACHE["nc"]


def kernel(**inputs) -> np.ndarray:
    nc = _get_nc()
    in_maps = host_prep(inputs)
    res = bass_utils.run_bass_kernel_spmd(nc, in_maps,
                                          core_ids=list(range(NCORES)))
    out = np.empty((B, 64, H, W), np.float32)
    for b in range(B):
        out[b] = res.results[2 * b]["outp"].reshape(64, H, W)
    return out


if __name__ == "__main__":
    rng = np.random.default_rng(0)
    d = {
        "xc": rng.standard_normal((B, C, H, W), np.float32),
        "xt": rng.standard_normal((B, HW, C), np.float32),
    }
    for nm, o in (("q_c", C), ("k_c", C), ("v_c", C), ("q_t", C), ("k_t", C)):
        d[f"W{nm}"] = rng.standard_normal((o, C), np.float32) * 0.02
        d[f"b{nm}"] = np.zeros(o, np.float32)
    d["W512_64"] = rng.standard_normal((64, C), np.float32) * 0.02
    d["b512_64"] = np.zeros(64, np.float32)
    for i in (1, 2, 3):
        d[f"W{i}"] = rng.standard_normal((64, 128, 3, 3), np.float32) * 0.02
        d[f"b{i}"] = np.zeros(64, np.float32)
    out = kernel(**d)
    print("out", out.shape, out.dtype, np.abs(out).max())


# revision 17
# speedup vs baseline: 1.3531x; 1.2110x over previous
"""Trainium2 Bass kernel for nn_MCM_37031208026850 (v17).

Strategy (8 NeuronCores, SPMD):
  - Folded score algebra: score_ij = q_i.k_j = qhat_i.x_j + a_i with
    qhat = (Wq^T Wk)^T x + Wk^T bq. The row-constant a_i and the exact
    mean-over-HW term are host matvecs folded into musum, so the device
    only needs max_j(qhat.x_j) per (row, batch).
  - The k side of every score GEMM is therefore the RAW input x, which the
    host replicates to all 8 cores in bf16 (resident in SBUF) - the two
    1MB-per-rank k AllGathers of v16 are gone. Only two tiny pair-wise
    co AllGathers (2x512 fp32 within each core pair) remain, since the
    softmax gate for batch b couples exactly the rows owned by cores
    2b/2b+1.
  - Scores run bf16 x bf16 (measured 913ns vs 988ns fp32r per 128x512
    tile), EXCEPT one of the four per-combo batch max terms (b == combo)
    which runs pure-fp8 DoubleRow (624ns/tile measured, 2 moving rows per
    cycle): all-fp8 scores would give rel_err 3.0e-2 > 2e-2, but 1-of-4
    halves the co noise to a measured 1.66e-2 (deterministic inputs).
    Error-compensated fp8 (q hi/lo pairs) measured slower than bf16 and
    was rejected. PSUM accumulates fp32; the DVE max-reduces [P,1024]
    groups (one per (mi, combo, batch)) directly into maxd.
  - Shard: core r owns global score rows [512r, 512(r+1)) (batch r//2).
    Projections, values, gating and the three 3x3 convs as in v16, with
    bf16 operands everywhere on the PE; conv PSUM groups cover 16 rows
    (512-wide) to halve instruction-overhead, and the qi=0 gate softmax
    runs inside the score loop so seg3 starts with T1 gating immediately.
  - Timing builds emit each segment body TIME_UNROLL times per For_i
    iteration so the loop-barrier cost is amortized in the measurement.
  - Measured (same-session A/B): v16 271.2us/iter -> v17 ~158-176us/iter
    (regime-dependent; lower-quartile paired-round slope reported);
    collective round (2 tiny AllGathers) 15.1us.
"""
import sys
sys.path.insert(0, "/opt/trn_rl_repo")

import numpy as np
import ml_dtypes

import concourse.bass as bass
import concourse.mybir as mybir
import concourse.tile as tile
from concourse import bacc
from concourse import bass_utils
from concourse.masks import make_identity

B, C, H, W = 4, 512, 32, 32
HW = H * W
SCALE = 1.0 / C ** 0.5
NCORES = 8
P = 128
KT = C // P          # 4 k-tiles over channels
S = 512              # q-rows per core
BHW = B * HW
TIME_UNROLL = 2      # bodies emitted per For_i iteration in timing builds
F32 = mybir.dt.float32
F32R = mybir.dt.float32r
BF16 = mybir.dt.bfloat16
AX = mybir.AxisListType.X
AF = mybir.ActivationFunctionType
MUL = mybir.AluOpType.mult
ADD = mybir.AluOpType.add

BF = ml_dtypes.bfloat16


def _kmaj(a):
    # (C, n) -> (P, KT, n) k-major layout matching SBUF tiles
    n = a.shape[1]
    return np.ascontiguousarray(
        a.reshape(KT, P, n).transpose(1, 0, 2))


# ----------------------------------------------------------------------------
# host-side preparation
# ----------------------------------------------------------------------------

def host_prep(inputs):
    """Build the 8 per-core input maps from the full problem inputs."""
    xc = np.ascontiguousarray(inputs["xc"], dtype=np.float32)
    xt = np.ascontiguousarray(inputs["xt"], dtype=np.float32)
    f = lambda k: np.ascontiguousarray(inputs[k], dtype=np.float32)
    Wq = [f("Wq_c"), f("Wq_t")]
    bq = [f("bq_c"), f("bq_t")]
    Wk = [f("Wk_c"), f("Wk_t")]
    bk = [f("bk_c"), f("bk_t")]
    Wv_c, bv_c = f("Wv_c"), f("bv_c")
    W64, b64 = f("W512_64"), f("b512_64")
    W1, b1 = f("W1"), f("b1")
    W2, b2 = f("W2"), f("b2")
    W3, b3 = f("W3"), f("b3")

    xcG = np.ascontiguousarray(
        xc.reshape(B, C, HW).transpose(1, 0, 2).reshape(C, BHW))
    xtT = np.ascontiguousarray(
        xt.transpose(2, 0, 1).reshape(C, BHW))
    Xs = [xcG, xtT]

    # exact mean term + row constant a_i, folded into musum:
    #   musum[i, combo] = sum_b mean_j(q_i.k_j) + 4*a_i
    #   with q = Wq x + bq, k = Wk x + bk, a_i = q_i . bk
    kbar = []
    for kk in range(2):
        ks = (Wk[kk] @ Xs[kk].reshape(C, B, HW).sum(-1)
              + HW * bk[kk][:, None]) / HW          # (C, B) per-batch k mean
        kbar.append(ks.sum(1))                      # (C,)
    mus_full = np.zeros((2, 2, BHW), np.float32)
    for qi in range(2):
        for kk in range(2):
            u = Wq[qi].T @ kbar[kk]
            mean_t = Xs[qi].T @ u + float(bq[qi] @ kbar[kk])
            a_i = Xs[qi].T @ (Wq[qi] @ bk[kk]) + float(bq[qi] @ bk[kk])
            mus_full[qi, kk] = mean_t + 4.0 * a_i

    # folded qhat projection weights: qhat = M^T x + c,  M = Wq^T Wk
    mblob = np.concatenate(
        [_kmaj((Wq[qi].T @ Wk[kk]).astype(np.float32))
         for qi in range(2) for kk in range(2)],
        axis=2).astype(BF)                          # (P, KT, 4*512)
    cbias = np.stack(
        [(Wk[kk].T @ bq[qi]).reshape(KT, P).T
         for qi in range(2) for kk in range(2)],
        axis=1)                                     # (P, 4, KT)

    # value weights (folded 64ch c-path), conv weights
    Wcv = W64 @ Wv_c
    bcv = W64 @ bv_c
    wcv64 = _kmaj(np.ascontiguousarray(np.concatenate([Wcv, Wcv], 0).T))
    wtv64 = _kmaj(np.ascontiguousarray(np.concatenate([W64, W64], 0).T))
    w3t = W3.transpose(1, 2, 3, 0).reshape(P, 9 * 64)
    pad = np.zeros((P - 64, 9 * 64), np.float32)
    vblob_shared = np.hstack([
        wcv64.reshape(P, KT * P), wtv64.reshape(P, KT * P),
        W1.transpose(1, 2, 3, 0).reshape(P, 9 * 64),
        W2.transpose(1, 2, 3, 0).reshape(P, 9 * 64),
        np.vstack([w3t[:64], pad]),
        np.vstack([w3t[64:], pad]),
    ]).astype(np.float32)                           # (P, 3328)

    cpad = np.zeros((P - 64, 1), np.float32)
    bblob = np.hstack([
        cbias.reshape(P, 16),
        np.concatenate([bcv, bcv]).reshape(P, 1),
        np.concatenate([b64, b64]).reshape(P, 1),
        np.vstack([b1.reshape(64, 1), cpad]),
        np.vstack([b2.reshape(64, 1), cpad]),
        np.vstack([b3.reshape(64, 1), cpad]),
    ]).astype(np.float32)                           # (P, 21)

    xk_c = _kmaj(xcG).astype(BF).reshape(P, KT * BHW)
    xk_t = _kmaj(xtT).astype(BF).reshape(P, KT * BHW)
    F8 = ml_dtypes.float8_e4m3
    xk8_c = np.concatenate(
        [_kmaj(xcG[:, HW * b:HW * (b + 1)]).astype(F8) for b in (0, 2)],
        axis=2).reshape(P, KT * 2 * HW)
    xk8_t = np.concatenate(
        [_kmaj(xtT[:, HW * b:HW * (b + 1)]).astype(F8) for b in (1, 3)],
        axis=2).reshape(P, KT * 2 * HW)
    shared = {
        "mblob": np.ascontiguousarray(mblob.reshape(P, KT * 4 * 512)),
        "bblob": np.ascontiguousarray(bblob),
        "xk_c": np.ascontiguousarray(xk_c),
        "xk_t": np.ascontiguousarray(xk_t),
        "xk8_c": np.ascontiguousarray(xk8_c),
        "xk8_t": np.ascontiguousarray(xk8_t),
    }

    in_maps = []
    for r in range(NCORES):
        myb = r // 2
        cols = slice(S * r, S * (r + 1))
        bcols = slice(HW * myb, HW * (myb + 1))
        # gate-row one-hots: gates_sb rows are [c2*4 + b]; T1 = [c_co; ct_co]
        # (combos 0,1 of half 0), T2 = [t_co; tc_co] (combos 3,2 of half 1)
        sel1 = np.zeros((8, P), np.float32)
        sel2 = np.zeros((8, P), np.float32)
        for p in range(P):
            sel1[(0 if p < 64 else 1) * 4 + myb, p] = 1.0
            sel2[((3 if p < 64 else 2) - 2) * 4 + myb, p] = 1.0
        selpad = np.zeros((P - 8, P), np.float32)
        vblob = np.hstack([
            vblob_shared,
            np.vstack([sel1, selpad]),
            np.vstack([sel2, selpad]),
        ]).astype(BF)                               # (P, 3584)
        mus = np.zeros((P, KT, 4), np.float32)
        for qi in range(2):
            for kk in range(2):
                mus[:, :, 2 * qi + kk] = (
                    mus_full[qi, kk, cols].reshape(KT, P).T)
        m = dict(shared)
        m["vblob"] = np.ascontiguousarray(vblob)
        m["musum"] = np.ascontiguousarray(mus.reshape(P, KT * 4))
        m["xq_c"] = np.ascontiguousarray(
            _kmaj(xcG[:, cols]).astype(BF).reshape(P, KT * S))
        m["xq_t"] = np.ascontiguousarray(
            _kmaj(xtT[:, cols]).astype(BF).reshape(P, KT * S))
        m["xv_c"] = np.ascontiguousarray(
            _kmaj(xcG[:, bcols]).astype(BF).reshape(P, KT * HW))
        m["xv_t"] = np.ascontiguousarray(
            _kmaj(xtT[:, bcols]).astype(BF).reshape(P, KT * HW))
        in_maps.append(m)
    return in_maps


# ----------------------------------------------------------------------------
# device program
# ----------------------------------------------------------------------------

def build_program(time_reps: int = 1):
    """Build + bacc-compile the SPMD Bass program.

    time_reps > 1 wraps the three compute segments in For_i loops
    (collectives stay outside) so wall-clock deltas between different reps
    counts measure pure per-iteration compute time.
    """
    import contextlib
    nc = bacc.Bacc("TRN2", target_bir_lowering=False, debug=False,
                   num_devices=NCORES)

    def din(name, shape, dtype):
        return nc.dram_tensor(name, list(shape), dtype, kind="ExternalInput")

    xq_d = [din("xq_c", (P, KT * S), BF16), din("xq_t", (P, KT * S), BF16)]
    xk_d = [din("xk_c", (P, KT * BHW), BF16),
            din("xk_t", (P, KT * BHW), BF16)]
    FP8 = mybir.dt.float8e4
    xk8_d = [din("xk8_c", (P, KT * 2 * HW), FP8),
             din("xk8_t", (P, KT * 2 * HW), FP8)]
    xv_d = [din("xv_c", (P, KT * HW), BF16), din("xv_t", (P, KT * HW), BF16)]
    mblob_d = din("mblob", (P, KT * 4 * 512), BF16)
    VBLOB = 512 + 512 + 576 + 576 + 576 + 576 + 128 + 128
    vblob_d = din("vblob", (P, VBLOB), BF16)
    bblob_d = din("bblob", (P, 21), F32)
    musum_d = din("musum", (P, KT * 4), F32)

    outp_d = nc.dram_tensor("outp", [64, HW], F32, kind="ExternalOutput")


    with tile.TileContext(nc) as tc:
      with tc.tile_pool(name="consts", bufs=1) as cons, \
           tc.tile_pool(name="dram", bufs=1, space="DRAM") as dram:
        # ---------------- static loads (phase-ordered) ----------------
        bblob_sb = cons.tile([P, 21], F32)
        nc.sync.dma_start(bblob_sb, bblob_d.ap())
        xq_sb = [cons.tile([P, KT, S], BF16, name=f"xq{i}") for i in range(2)]
        mblob_sb = cons.tile([P, KT, 4, 512], BF16)
        mb_view = mblob_d.ap().rearrange("p (kt c n) -> p kt c n", kt=KT, c=4)
        for i in range(2):
            nc.sync.dma_start(
                xq_sb[i], xq_d[i].ap().rearrange("p (kt n) -> p kt n", kt=KT))
        for kt in range(KT):
            nc.sync.dma_start(mblob_sb[:, kt], mb_view[:, kt])
        musum_sb = cons.tile([P, KT, 4], F32)
        nc.sync.dma_start(musum_sb, musum_d.ap().rearrange(
            "p (kt c) -> p kt c", kt=KT))
        # k-side raw x, resident bf16, loaded in batch-chunks so the first
        # score groups wait only on their own chunk
        xk_sb = [cons.tile([P, KT, BHW], BF16, name=f"xk{i}")
                 for i in range(2)]
        for i in range(2):
            xk_view = xk_d[i].ap().rearrange("p (kt n) -> p kt n", kt=KT)
            for b in range(B):
                cs = slice(HW * b, HW * (b + 1))
                nc.sync.dma_start(xk_sb[i][:, :, cs], xk_view[:, :, cs])
        xk8_sb = [cons.tile([P, KT, 2 * HW], FP8, name=f"xk8{i}")
                  for i in range(2)]
        for i in range(2):
            nc.sync.dma_start(
                xk8_sb[i],
                xk8_d[i].ap().rearrange("p (kt n) -> p kt n", kt=KT))
        vblob_sb = cons.tile([P, VBLOB], BF16)
        nc.sync.dma_start(vblob_sb, vblob_d.ap())
        xv_sb = [cons.tile([P, KT, HW], BF16, name=f"xv{i}") for i in range(2)]
        for i in range(2):
            nc.sync.dma_start(
                xv_sb[i], xv_d[i].ap().rearrange("p (kt n) -> p kt n", kt=KT))

        # blob views
        def vsl(lo, n):
            return vblob_sb[:, lo:lo + n]

        wcv_sb = vsl(0, 512).rearrange("p (kt n) -> p kt n", kt=KT)
        wtv_sb = vsl(512, 512).rearrange("p (kt n) -> p kt n", kt=KT)
        w1t_sb = vsl(1024, 576).rearrange("p (t n) -> p t n", t=9)
        w2t_sb = vsl(1600, 576).rearrange("p (t n) -> p t n", t=9)
        w3a_sb = vblob_sb[0:64, 2176:2752].rearrange("p (t n) -> p t n", t=9)
        w3b_sb = vblob_sb[0:64, 2752:3328].rearrange("p (t n) -> p t n", t=9)
        sel1_sb = vblob_sb[0:8, 3328:3456]
        sel2_sb = vblob_sb[0:8, 3456:3584]
        conv_w = [w1t_sb, w2t_sb]
        cb_sb = bblob_sb[:, 0:16].rearrange("p (c kt) -> p c kt", c=4)
        bcv_sb = bblob_sb[:, 16:17]
        b64_sb = bblob_sb[:, 17:18]
        conv_b = [bblob_sb[0:64, 18:19], bblob_sb[0:64, 19:20]]
        cb3_sb = bblob_sb[0:64, 20:21]

        ident = cons.tile([P, P], F32)
        make_identity(nc, ident)

        # persistent intermediates
        q_sb = [cons.tile([P, KT, S], BF16, name=f"q{i}") for i in range(4)]
        q8_sb = [cons.tile([P, KT, S], FP8, name=f"q8{i}") for i in range(4)]
        cv_sb = cons.tile([P, HW], F32)
        tv_sb = cons.tile([P, HW], F32)
        maxd = cons.tile([P, KT, 4, 4], F32)       # [i, mi, combo, b]
        sumd = cons.tile([P, KT, 4], F32)
        co_sb = cons.tile([P, KT, 4], F32)         # [i, mi, combo]
        co_row = [cons.tile([2, S], F32, name=f"cor{i}") for i in range(2)]
        gates_sb = [cons.tile([8, HW], F32, name=f"g{i}") for i in range(2)]
        rmax = [cons.tile([8, 1], F32, name=f"rm{i}") for i in range(2)]
        negmax = [cons.tile([8, 1], F32, name=f"nm{i}") for i in range(2)]
        expacc = [cons.tile([8, 1], F32, name=f"ea{i}") for i in range(2)]
        rsum = [cons.tile([8, 1], F32, name=f"rs{i}") for i in range(2)]
        expg = [cons.tile([8, HW], F32, name=f"eg{i}") for i in range(2)]
        gates_n = [cons.tile([8, HW], BF16, name=f"gn{i}") for i in range(2)]
        T1 = cons.tile([P, H + 2, W + 2], BF16)
        T2 = cons.tile([P, H + 2, W + 2], BF16)
        T3a = cons.tile([64, H + 2, W + 2], BF16)
        T3b = cons.tile([64, H + 2, W + 2], BF16)
        out_sb = cons.tile([64, H, W], F32)
        for T in (T1, T2, T3a, T3b):
            nc.vector.memset(T.bitcast(mybir.dt.uint16), 0)

        co_dram = [dram.tile([2, S], F32, name=f"cod{i}") for i in range(2)]
        co_all = [dram.tile([NCORES * 2, S], F32, addr_space="Shared",
                            name=f"coa{i}") for i in range(2)]

        rep = (lambda: tc.For_i(0, time_reps, 1)) if time_reps > 1 else None

        # ------------- segment 1+2: projections, scores + co -------------
        with tc.tile_pool(name="pj", bufs=2, space="PSUM") as pj, \
             tc.tile_pool(name="sc", bufs=2, space="PSUM") as sc, \
             tc.tile_pool(name="vp", bufs=1, space="PSUM") as vp, \
             tc.tile_pool(name="fin", bufs=1, space="PSUM") as fin:
          with rep() if rep else contextlib.nullcontext():
           for _u in range(TIME_UNROLL if time_reps > 1 else 1):
            evac_i = 0
            for combo in range(4):
                qi = combo // 2
                for m in range(KT):
                    pq = pj.tile([P, S], F32, tag="pq", name="pq")
                    for kt in range(KT):
                        nc.tensor.matmul(
                            pq,
                            mblob_sb[:, kt, combo, P * m:P * (m + 1)],
                            xq_sb[qi][:, kt], start=(kt == 0),
                            stop=(kt == KT - 1))
                    if evac_i % 2 == 0:
                        nc.vector.tensor_scalar_add(
                            q_sb[combo][:, m, :], pq, cb_sb[:, combo, m:m + 1])
                        nc.scalar.activation(
                            q8_sb[combo][:, m, :], pq, AF.Identity,
                            bias=cb_sb[:, combo, m:m + 1])
                    else:
                        nc.scalar.activation(
                            q_sb[combo][:, m, :], pq, AF.Identity,
                            bias=cb_sb[:, combo, m:m + 1])
                        nc.vector.tensor_scalar_add(
                            q8_sb[combo][:, m, :], pq, cb_sb[:, combo, m:m + 1])
                    evac_i += 1

            DRM = mybir.MatmulPerfMode.DoubleRow

            def score_group(combo, b, mi):
                kk = combo % 2
                ps = sc.tile([P, 1024], F32, tag="ps", name="ps")
                if b == combo:
                    # this batch's max runs in pure-fp8 DoubleRow (1 of 4
                    # max terms per combo; rel-err budget measured on
                    # device). xk8 column base: batch slot b//2.
                    c0 = HW * (b // 2)
                    for cb4 in range(4):
                        for slab in range(2):
                            nc.tensor.matmul(
                                ps[:, 256 * cb4:256 * (cb4 + 1)],
                                q8_sb[combo][:, 2 * slab:2 * slab + 2,
                                             P * mi:P * (mi + 1)],
                                xk8_sb[kk][:, 2 * slab:2 * slab + 2,
                                           c0 + 256 * cb4:
                                           c0 + 256 * (cb4 + 1)],
                                start=(slab == 0), stop=(slab == 1),
                                perf_mode=DRM)
                else:
                    for h_ in range(2):
                        for kt in range(KT):
                            nc.tensor.matmul(
                                ps[:, 512 * h_:512 * (h_ + 1)],
                                q_sb[combo][:, kt, P * mi:P * (mi + 1)],
                                xk_sb[kk][:, kt,
                                          HW * b + 512 * h_:
                                          HW * b + 512 * (h_ + 1)],
                                start=(kt == 0), stop=(kt == KT - 1))
                nc.vector.reduce_max(
                    maxd[:, mi, combo, b:b + 1], ps, axis=AX)

            def co_half(qi):
                sl = slice(2 * qi, 2 * qi + 2)
                nc.vector.reduce_sum(sumd[:, :, sl], maxd[:, :, sl], axis=AX)
                nc.vector.tensor_tensor(co_sb[:, :, sl], sumd[:, :, sl],
                                        musum_sb[:, :, sl], ADD)
                for mi in range(KT):
                    ptr = fin.tile([P, P], F32, tag="ptr", name="ptr")
                    nc.tensor.transpose(ptr[:2, :], co_sb[:, mi, sl], ident)
                    nc.vector.tensor_copy(
                        co_row[qi][:, P * mi:P * (mi + 1)], ptr[:2, :])
                nc.sync.dma_start(co_dram[qi].opt(), co_row[qi])
                if time_reps <= 1:
                    nc.gpsimd.collective_compute(
                        "AllGather", mybir.AluOpType.bypass,
                        replica_groups=[list(range(NCORES))],
                        ins=[co_dram[qi].opt()], outs=[co_all[qi].opt()])

            def gates_half(qi):
                # gates rows (cmb in half qi) x batch, softmaxed
                co_view = co_all[qi].opt().rearrange(
                    "(b h c) i -> c b h i", b=4, h=2, c=2)
                for c2 in range(2):
                    nc.sync.dma_start(
                        gates_sb[qi][4 * c2:4 * (c2 + 1), :].rearrange(
                            "p (h i) -> p h i", h=2),
                        co_view[c2])
                nc.vector.reduce_max(rmax[qi], gates_sb[qi], axis=AX)
                nc.vector.tensor_scalar_mul(negmax[qi], rmax[qi], -SCALE)
                nc.scalar.activation(expg[qi], gates_sb[qi], AF.Exp,
                                     bias=negmax[qi], scale=SCALE,
                                     accum_out=expacc[qi])
                nc.vector.reciprocal(rsum[qi], expacc[qi])
                nc.vector.tensor_scalar_mul(gates_n[qi], expg[qi], rsum[qi])

            for qi in range(2):
                for combo in (2 * qi, 2 * qi + 1):
                    for b in range(B):
                        for mi in range(KT):
                            score_group(combo, b, mi)
                if qi == 1:
                    # folded 64-ch value projections fill the PE drain gap
                    for vi, (wv, vt) in enumerate(((wcv_sb, cv_sb),
                                                   (wtv_sb, tv_sb))):
                        for nh in range(2):
                            pv = vp.tile([P, 512], F32, tag="pv", name="pv")
                            for kt in range(KT):
                                nc.tensor.matmul(
                                    pv, wv[:, kt],
                                    xv_sb[vi][:, kt,
                                              512 * nh:512 * (nh + 1)],
                                    start=(kt == 0), stop=(kt == KT - 1))
                            if vi == 0:
                                nc.scalar.activation(
                                    vt[:, 512 * nh:512 * (nh + 1)], pv,
                                    AF.Identity, bias=bcv_sb)
                            else:
                                nc.scalar.copy(
                                    vt[:, 512 * nh:512 * (nh + 1)], pv)
                co_half(qi)
                if qi == 0:
                    gates_half(0)

        if time_reps > 1:
            for qi in range(2):
                nc.gpsimd.collective_compute(
                    "AllGather", mybir.AluOpType.bypass,
                    replica_groups=[list(range(NCORES))],
                    ins=[co_dram[qi].opt()], outs=[co_all[qi].opt()])

        # ---------------- segment 3: gates + fusion convs ----------------
        with tc.tile_pool(name="g", bufs=2, space="PSUM") as g:
          with rep() if rep else contextlib.nullcontext():
           for _u in range(TIME_UNROLL if time_reps > 1 else 1):
            def gate_one(sel, gn, val, T):
                for nh in range(2):
                    pbg = g.tile([P, 512], F32, tag="pbg", name="pbg")
                    nc.tensor.matmul(pbg, sel,
                                     gn[:, 512 * nh:512 * (nh + 1)],
                                     start=True, stop=True)
                    reg = T[:, 1 + 16 * nh:17 + 16 * nh, 1:33]
                    nc.vector.tensor_tensor(
                        reg, pbg.rearrange("p (y x) -> p y x", y=16),
                        val[:, 512 * nh:512 * (nh + 1)].rearrange(
                            "p (y x) -> p y x", y=16), MUL)
                    nc.vector.tensor_scalar_add(reg, reg, b64_sb)

            def conv12(srcT, wi, dstT):
                for cy in range(2):
                    pc = g.tile([64, 16, 32], F32, tag="pc", name="pc")
                    for tap in range(9):
                        dy, dx = tap // 3, tap % 3
                        nc.tensor.matmul(
                            pc, conv_w[wi][:, tap, :],
                            srcT[:, 16 * cy + dy:16 * cy + dy + 16,
                                 dx:dx + 32],
                            start=(tap == 0), stop=(tap == 8))
                    nc.scalar.activation(
                        dstT[:, 1 + 16 * cy:17 + 16 * cy, 1:33], pc, AF.Relu,
                        bias=conv_b[wi], scale=1.0)

            gate_one(sel1_sb, gates_n[0], cv_sb, T1)
            gates_half(1)
            conv12(T1, 0, T3a)
            gate_one(sel2_sb, gates_n[1], tv_sb, T2)
            conv12(T2, 1, T3b)
            for cy in range(2):
                pc = g.tile([64, 16, 32], F32, tag="pc", name="pc")
                for hi, (wh, Th) in enumerate(((w3a_sb, T3a), (w3b_sb, T3b))):
                    for tap in range(9):
                        dy, dx = tap // 3, tap % 3
                        nc.tensor.matmul(
                            pc, wh[:, tap, :],
                            Th[:, 16 * cy + dy:16 * cy + dy + 16,
                               dx:dx + 32],
                            start=(hi == 0 and tap == 0),
                            stop=(hi == 1 and tap == 8))
                nc.scalar.activation(out_sb[:, 16 * cy:16 * (cy + 1), :], pc,
                                     AF.Relu, bias=cb3_sb, scale=1.0)
                nc.sync.dma_start(
                    outp_d.ap().rearrange("o (y x) -> o y x",
                                          y=H)[:, 16 * cy:16 * (cy + 1), :],
                    out_sb[:, 16 * cy:16 * (cy + 1), :])

    nc.compile()
    return nc


# ----------------------------------------------------------------------------
# entry point
# ----------------------------------------------------------------------------

_CACHE = {}


def _get_nc():
    if "nc" not in _CACHE:
        _CACHE["nc"] = build_program()
    return _CACHE["nc"]


def kernel(**inputs) -> np.ndarray:
    nc = _get_nc()
    in_maps = host_prep(inputs)
    res = bass_utils.run_bass_kernel_spmd(nc, in_maps,
                                          core_ids=list(range(NCORES)))
    out = np.empty((B, 64, H, W), np.float32)
    for b in range(B):
        out[b] = res.results[2 * b]["outp"].reshape(64, H, W)
    return out


if __name__ == "__main__":
    rng = np.random.default_rng(0)
    d = {
        "xc": rng.standard_normal((B, C, H, W), np.float32),
        "xt": rng.standard_normal((B, HW, C), np.float32),
    }
    for nm, o in (("q_c", C), ("k_c", C), ("v_c", C), ("q_t", C), ("k_t", C)):
        d[f"W{nm}"] = rng.standard_normal((o, C), np.float32) * 0.02
        d[f"b{nm}"] = np.zeros(o, np.float32)
    d["W512_64"] = rng.standard_normal((64, C), np.float32) * 0.02
    d["b512_64"] = np.zeros(64, np.float32)
    for i in (1, 2, 3):
        d[f"W{i}"] = rng.standard_normal((64, 128, 3, 3), np.float32) * 0.02
        d[f"b{i}"] = np.zeros(64, np.float32)
    out = kernel(**d)
    print("out", out.shape, out.dtype, np.abs(out).max())
